# revision 43
# baseline (speedup 1.0000x reference)
"""Trainium2 Bass kernel for an 8-batch transformer encoder block.

Strategy: pure data parallelism -- batch B=8 across 8 NeuronCores, one
batch element (1024 tokens x 1024 dim) per core, full weights on every
core, no collectives.  All matmuls run in bf16 on the TensorEngine with
f32 PSUM accumulation; LayerNorm / softmax statistics stay f32 (weights
are pre-cast to bf16 on the host).

Layout notes (per core):
  - LayerNorm runs token-major; its bf16 output is flipped to the
    feature-major [C, tokens] layout the linears need via PE transposes
    (128x128 identity matmuls, free while PE is otherwise idle).
  - q/k projections emit feature-major qT/kT (per-partition bias fused
    into the PSUM->SBUF ACT copy); v emits token-major into SBUF.
  - The reference reshapes (N, C) -> (H, N', hd) directly, so head h of
    q/k/v is the contiguous row-block [64h, 64h+64) of the projection
    reinterpreted as (1024, 64).  Per-head Q^T/K^T tiles are gathered
    with two contiguous partition-shifted SBUF->SBUF DMAs each (one per
    column parity); V chunks are strided SBUF->SBUF reads.
  - Softmax is computed on S^T (keys on partitions) with no max
    subtraction (logits are ~N(0, 0.3), |logit| < ~7, exp safe in f32).
    The denominators come free from a ones-column appended to V in the
    P@V matmul; normalization is a per-column scale of the 64-row O^T
    head output (reciprocal on DVE, partition-broadcast on GpSimd).
  - MLP: FC1 emits hidden-major m1T with exact-erf GELU + bias fused in
    the ACT PSUM->SBUF copy; FC2 accumulates over 4 hidden blocks into
    an f32 SBUF accumulator, then adds bias + residual and stores.
  - PSUM: three 2-bank [128, 1024] matmul slots (two 512-wide matmuls
    per slot, one wide ACT/DVE drain) + two transpose banks.

Measured (8 cores, NeuronCore per batch element): ~0.64 ms/block,
TimelineSim model 0.60 ms; rel_l2 vs f32 reference = 1.5e-3 (bf16
floor).  KERNEL_NREP / KERNEL_DEBUG_TAPS env vars exist for the test
harness only (timing slope / intermediate taps).
"""

import os
import sys

sys.path.insert(0, "/opt/trn_rl_repo")

import numpy as np
import ml_dtypes

import concourse.bass as bass
import concourse.tile as tile
from concourse import bacc, mybir
from concourse.masks import make_identity

B, N, C, H = 8, 1024, 1024, 16
HD = C // H  # 64
HID = 4 * C  # 4096
P = 128
NT = N // P  # token chunks
CO = C // P  # feature chunks
JH = HID // P  # hidden chunks
EPS = 1e-5

F32 = mybir.dt.float32
BF16 = mybir.dt.bfloat16
AF = mybir.ActivationFunctionType
ALU = mybir.AluOpType

NCORES = 8

WEIGHT_NAMES = ["wq", "wk", "wv", "wp", "w1", "w2"]
VEC_NAMES = ["g1", "b1", "bq", "bk", "bv", "bp", "g2", "b2", "c1", "c2"]


def _ts(i, size):
    return slice(i * size, (i + 1) * size)


class _Pool:
    """Tile pool with manually controlled (non-LIFO) lifetime."""

    def __init__(self, tc, **kw):
        self._cm = tc.tile_pool(**kw)
        self.pool = self._cm.__enter__()

    _n = 0

    def tile(self, *a, **kw):
        if "name" not in kw:
            _Pool._n += 1
            kw["name"] = f"t{_Pool._n}"
        return self.pool.tile(*a, **kw)

    def close(self):
        self._cm.__exit__(None, None, None)


def build_program(nc):
    d = {}
    d["x"] = nc.dram_tensor("x", [N, C], F32, kind="ExternalInput").ap()
    for w, shape in [
        ("wq", [C, C]),
        ("wk", [C, C]),
        ("wv", [C, C]),
        ("wp", [C, C]),
        ("w1", [C, HID]),
        ("w2", [HID, C]),
    ]:
        d[w] = nc.dram_tensor(w, shape, BF16, kind="ExternalInput").ap()
    for v in VEC_NAMES:
        size = HID if v == "c1" else C
        d[v] = nc.dram_tensor(v, [size], F32, kind="ExternalInput").ap()
    d["out"] = nc.dram_tensor("out", [N, C], F32, kind="ExternalOutput").ap()

    debug = bool(os.environ.get("KERNEL_DEBUG_TAPS"))
    dbg = {}
    if debug:
        for nm, shape, dt in [
            ("dbg_h", [N, C], BF16),
            ("dbg_v", [N, C], BF16),
            ("dbg_h2", [N, C], BF16),
            ("dbg_qT", [P, CO, N], BF16),
            ("dbg_kT", [P, CO, N], BF16),
            ("dbg_oT", [P, CO, N], BF16),
            ("dbg_x1", [P, NT, C], F32),
        ]:
            dbg[nm] = nc.dram_tensor(nm, shape, dt, kind="ExternalOutput").ap()

    nrep = int(os.environ.get("KERNEL_NREP", "1"))
    with tile.TileContext(nc) as tc:
        for rep in range(nrep):
            _emit(tc, nc, d, dbg if rep == 0 else {})
    return nc


def _emit(tc, nc, d, dbg=None):
    dbg = dbg or {}
    # ------- resident pools (level 0): tiny consts, psum, x1 -------
    consts = _Pool(tc, name="consts", bufs=1)
    bq_sb = consts.tile([P, CO], F32)
    nc.sync.dma_start(bq_sb[:], d["bq"].rearrange("(o p) -> p o", p=P))
    bk_sb = consts.tile([P, CO], F32)
    nc.sync.dma_start(bk_sb[:], d["bk"].rearrange("(o p) -> p o", p=P))
    c1_sb = consts.tile([P, JH], F32)
    nc.sync.dma_start(c1_sb[:], d["c1"].rearrange("(j p) -> p j", p=P))
    eps_sb = consts.tile([P, 1], F32)
    nc.vector.memset(eps_sb[:], EPS)
    ident = consts.tile([P, P], BF16, name="ident")
    make_identity(nc, ident[:])

    # shared PSUM pool: all psum tiles are [128, 512] f32 (one bank each)
    psum = _Pool(tc, name="psum", bufs=6, space="PSUM")

    def ps_tile():
        return psum.tile([P, N], F32, tag="mm", name="ps", bufs=3)

    def ps_tr():
        # one PSUM bank holds eight 128x128 bf16 transposes -> one wide drain
        return psum.tile([P, CO, P], BF16, tag="tr", name="pstr", bufs=2)

    x1_pool = _Pool(tc, name="x1", bufs=1)
    x1 = x1_pool.tile([P, NT, C], F32)

    def rep_tile(pool, vname):
        t = pool.tile([P, C], F32, tag=f"{vname}_rep", name=f"{vname}_rep", bufs=1)
        nc.scalar.dma_start(t[:], d[vname].partition_broadcast(P))
        return t

    # ------- LayerNorm helper -------
    def layer_norm(work, src_ap, g_rep, b_rep, t):
        st = work.tile([P, 2, 6], F32, tag="ln_st", name="st")
        nc.vector.bn_stats(st[:, 0, :], src_ap[:, 0:512])
        nc.vector.bn_stats(st[:, 1, :], src_ap[:, 512:1024])
        mv = work.tile([P, 2], F32, tag="ln_mv", name="mv")
        nc.vector.bn_aggr(mv[:], st[:])
        rstd = work.tile([P, 1], F32, tag="ln_rstd", name="rstd")
        nc.scalar.activation(rstd[:], mv[:, 1:2], AF.Sqrt, bias=eps_sb[:, :])
        nc.vector.reciprocal(rstd[:], rstd[:])
        # (x - m)*g*rstd + b fused as two scalar_tensor_tensor passes
        tmp = work.tile([P, C], F32, tag="ln_tmp", name="tmp")
        nc.vector.scalar_tensor_tensor(
            tmp[:], src_ap, mv[:, 0:1], g_rep[:], op0=ALU.subtract, op1=ALU.mult
        )
        hb = work.tile([P, C], BF16, tag="ln_out", name="hb")
        nc.vector.scalar_tensor_tensor(
            hb[:], tmp[:], rstd[:], b_rep[:], op0=ALU.mult, op1=ALU.add
        )
        return hb

    # ------- level 1 (pool stack, outermost first): h2T (lives to FC1),
    # wp/oT (live through proj), qkT (through attention), vwork (v_sb,
    # through attention), hT (through QKV) -------
    h2T_pool = _Pool(tc, name="h2T", bufs=1)
    h2T = h2T_pool.tile([P, CO, N], BF16)
    wp_pool = _Pool(tc, name="wp", bufs=1)
    wp_sb = wp_pool.tile([P, CO, C], BF16)
    oT_pool = _Pool(tc, name="oT", bufs=1)
    oT = oT_pool.tile([P, CO, N], BF16)
    qkT_pool = _Pool(tc, name="qkT", bufs=1)
    qTh = [qkT_pool.tile([P, CO, 512], BF16, name=f"qT{i}") for i in range(2)]
    kTh = [qkT_pool.tile([P, CO, 512], BF16, name=f"kT{i}") for i in range(2)]

    # v buffer lives through attention; opened before hT for LIFO order
    vwork = _Pool(tc, name="vwork", bufs=1)
    v_sb = vwork.tile([P, NT, C], BF16, name="v_sb")
    bv_rep = rep_tile(vwork, "bv")

    # ------- phase 1+2: LN1 -> PE-transpose -> hT (bf16, feature-major) ---
    hT_pool = _Pool(tc, name="hT", bufs=1)
    hT = hT_pool.tile([P, CO, N], BF16)
    ln1 = _Pool(tc, name="ln1", bufs=3)
    g1_rep = rep_tile(ln1, "g1")
    b1_rep = rep_tile(ln1, "b1")
    for t in range(NT):
        xt = ln1.tile([P, C], F32, tag="ln_x", name="xt")
        nc.sync.dma_start(xt[:], d["x"][_ts(t, P), :])
        hb = layer_norm(ln1, xt[:], g1_rep, b1_rep, t)
        ptr = ps_tr()
        for o in range(CO):
            nc.tensor.transpose(ptr[:, o, :], hb[:, _ts(o, P)], ident[:])
        nc.scalar.copy(hT[:, :, _ts(t, P)], ptr[:])

    ln1.close()

    # ------- phase 3: QKV projections -------
    wqkv = _Pool(tc, name="wqkv", bufs=1)
    w_sb = {}
    for w in ["wq", "wk", "wv"]:
        w_sb[w] = wqkv.tile([P, CO, C], BF16, name=f"{w}_sb")
        nc.sync.dma_start(w_sb[w][:], d[w].rearrange("(o p) c -> p o c", p=P))
    nc.sync.dma_start(wp_sb[:], d["wp"].rearrange("(o p) c -> p o c", p=P))

    # q/k in two token-half passes (heads 0-7 need only tokens 0..511, so
    # attention exp can start while the second half still projects); two
    # feature-chunks m share one wide psum slot.  v per token chunk.
    for half in range(2):
        hslice = slice(512 * half, 512 * (half + 1))
        for w, b_sb, dstT in (("wq", bq_sb, qTh), ("wk", bk_sb, kTh)):
            for mp in range(CO // 2):
                ps = ps_tile()
                for o in range(CO):
                    for mm in range(2):
                        m = 2 * mp + mm
                        nc.tensor.matmul(
                            ps[:, _ts(mm, 512)],
                            w_sb[w][:, o, _ts(m, P)],
                            hT[:, o, hslice],
                            start=(o == 0),
                            stop=(o == CO - 1),
                        )
                for mm in range(2):
                    m = 2 * mp + mm
                    nc.scalar.activation(
                        dstT[half][:, m, :],
                        ps[:, _ts(mm, 512)],
                        AF.Identity,
                        bias=b_sb[:, m : m + 1],
                    )
        # v token chunks of this half
        for t in range(4 * half, 4 * half + 4):
            ps = ps_tile()
            for o in range(CO):
                lhsT = hT[:, o, _ts(t, P)]
                nc.tensor.matmul(
                    ps[:, 0:512], lhsT, w_sb["wv"][:, o, 0:512],
                    start=(o == 0), stop=(o == CO - 1),
                )
                nc.tensor.matmul(
                    ps[:, 512:1024], lhsT, w_sb["wv"][:, o, 512:1024],
                    start=(o == 0), stop=(o == CO - 1),
                )
            nc.vector.tensor_tensor(v_sb[:, t, :], ps[:], bv_rep[:], op=ALU.add)
    if dbg:
        for o in range(CO):
            nc.sync.dma_start(
                dbg["dbg_h"][:, _ts(o, P)].rearrange("n c -> c n"), hT[:, o, :]
            )
        nc.sync.dma_start(
            dbg["dbg_v"].rearrange("(t p) c -> p t c", p=P), v_sb[:]
        )
        for i in range(2):
            nc.sync.dma_start(dbg["dbg_qT"][:, :, _ts(i, 512)], qTh[i][:])
            nc.sync.dma_start(dbg["dbg_kT"][:, :, _ts(i, 512)], kTh[i][:])
    wqkv.close()
    hT_pool.close()

    # ------- phase 4: attention, head by head -------
    heads = _Pool(tc, name="heads", bufs=2)
    for h in range(H):
        # Q_h^T / K_h^T as [64 d, (16 beta, 64 alpha)]; attention position
        # n = 16*alpha + beta.  Source: qT[64*beta + dd, 64h + alpha].
        qh = heads.tile([HD, 16, HD], BF16, tag="qh", name="qh")
        kh = heads.tile([HD, 16, HD], BF16, tag="kh", name="kh")
        hv = h % 8  # token offset within the half tile
        for srcT, dstT in ((qTh[h // 8], qh), (kTh[h // 8], kh)):
            # all b of one parity in a single DMA: b = 2o + bb
            for bb in range(2):
                nc.sync.dma_start(
                    dstT[:, bb::2, :],
                    srcT[64 * bb : 64 * bb + HD, :, _ts(hv, HD)],
                )
        # V_h chunks + ones column for softmax denominators.  Chunk i holds
        # m-values with m%16 in {2i, 2i+1} at partition p = 64*bb + a'
        # (m = 16a' + 2i + bb), matching the S^T psum partition order below.
        vh = heads.tile([P, 8, HD + 1], BF16, tag="vh", name="vh")
        nc.vector.memset(vh[:, :, HD : HD + 1], 1.0)
        # v rows 64h..64h+64 live at partitions 64*(h%2).. of chunk h//2
        vrow = v_sb[64 * (h % 2) : 64 * (h % 2) + 64, h // 2, :].rearrange(
            "t (g dd) -> t g dd", dd=HD
        )
        for bb in range(2):
            nc.sync.dma_start(
                vh[64 * bb : 64 * bb + 64, :, 0:HD], vrow[:, bb::2, :]
            )

        # S^T = K_h Q_h^T (keys on partitions), exp via ACT (scale=1/8).
        # psum partition p = 64*(b'%2) + a' <-> m = 16a' + 2i + b'%2.
        est = heads.tile([P, 8, N], BF16, tag="est", name="est")
        for i in range(8):
            ps = ps_tile()
            lhsT = kh[:, 2 * i : 2 * i + 2, :]  # [64, 128]
            nc.tensor.matmul(
                ps[:, 0:512], lhsT, qh[:, 0:8, :], start=True, stop=True
            )
            nc.tensor.matmul(
                ps[:, 512:1024], lhsT, qh[:, 8:16, :], start=True, stop=True
            )
            nc.scalar.activation(est[:, i, :], ps[:], AF.Exp, scale=0.125)

        # O^T = [V|1]^T expS^T : rows 0..63 head output, row 64 denominators
        po = ps_tile()
        for i in range(8):
            nc.tensor.matmul(
                po[0 : HD + 1, 0:512],
                vh[:, i, :],
                est[:, i, 0:512],
                start=(i == 0),
                stop=(i == 7),
            )
            nc.tensor.matmul(
                po[0 : HD + 1, 512:1024],
                vh[:, i, :],
                est[:, i, 512:1024],
                start=(i == 0),
                stop=(i == 7),
            )
        r = heads.tile([1, N], F32, tag="r", name="r")
        nc.vector.reciprocal(r[:], po[HD : HD + 1, :])
        rr = heads.tile([HD, N], F32, tag="rr", name="rr")
        nc.gpsimd.partition_broadcast(rr[:], r[:], channels=HD)

        # normalize + un-permute (beta, alpha) -> n = 16*alpha + beta
        p0 = HD * (h % 2)
        oc = h // 2
        for half in range(2):
            dst = oT[p0 : p0 + HD, oc, :].rearrange("p (a b2) -> p b2 a", b2=16)[
                :, 8 * half : 8 * half + 8, :
            ]
            src_ps = po[0:HD, _ts(half, 512)].rearrange("p (b2 a) -> p b2 a", b2=8)
            src_rr = rr[:, _ts(half, 512)].rearrange("p (b2 a) -> p b2 a", b2=8)
            nc.vector.tensor_tensor(dst, src_ps, src_rr, op=ALU.mult)
    heads.close()
    vwork.close()
    qkT_pool.close()
    if dbg:
        nc.sync.dma_start(dbg["dbg_oT"], oT[:])

    # ------- phase 5: proj + residual -> x1 ; LN2 -> h2T (PE transpose) ---
    ln2 = _Pool(tc, name="ln2", bufs=3)
    g2_rep = rep_tile(ln2, "g2")
    b2_rep = rep_tile(ln2, "b2")
    bp_rep = rep_tile(ln2, "bp")
    for t in range(NT):
        ps = ps_tile()
        for o in range(CO):
            lhsT = oT[:, o, _ts(t, P)]
            nc.tensor.matmul(
                ps[:, 0:512], lhsT, wp_sb[:, o, 0:512],
                start=(o == 0), stop=(o == CO - 1),
            )
            nc.tensor.matmul(
                ps[:, 512:1024], lhsT, wp_sb[:, o, 512:1024],
                start=(o == 0), stop=(o == CO - 1),
            )
        xt = ln2.tile([P, C], F32, tag="ln_x", name="xt")
        nc.sync.dma_start(xt[:], d["x"][_ts(t, P), :])
        nc.vector.tensor_tensor(x1[:, t, :], ps[:], bp_rep[:], op=ALU.add)
        nc.vector.tensor_tensor(x1[:, t, :], x1[:, t, :], xt[:], op=ALU.add)
        hb2 = layer_norm(ln2, x1[:, t, :], g2_rep, b2_rep, t)
        ptr = ps_tr()
        for o in range(CO):
            nc.tensor.transpose(ptr[:, o, :], hb2[:, _ts(o, P)], ident[:])
        nc.scalar.copy(h2T[:, :, _ts(t, P)], ptr[:])
    ln2.close()
    oT_pool.close()
    wp_pool.close()
    if dbg:
        nc.sync.dma_start(dbg["dbg_x1"], x1[:])
        for o in range(CO):
            nc.sync.dma_start(
                dbg["dbg_h2"][:, _ts(o, P)].rearrange("n c -> c n"), h2T[:, o, :]
            )

    # ------- phase 7: FC1 + exact GELU -> m1T -------
    m1_pool = _Pool(tc, name="m1T", bufs=1)
    m1T = m1_pool.tile([P, JH, N], BF16)
    w1s = _Pool(tc, name="w1s", bufs=3)
    w1_r = d["w1"].rearrange("(o p) c -> p o c", p=P)
    for j in range(JH):
        w1t = w1s.tile([P, CO, P], BF16, tag="w1t", name="w1t")
        nc.scalar.dma_start(w1t[:], w1_r[:, :, _ts(j, P)])
        ps = ps_tile()
        for o in range(CO):
            nc.tensor.matmul(
                ps[:, 0:512], w1t[:, o, :], h2T[:, o, 0:512],
                start=(o == 0), stop=(o == CO - 1),
            )
            nc.tensor.matmul(
                ps[:, 512:1024], w1t[:, o, :], h2T[:, o, 512:1024],
                start=(o == 0), stop=(o == CO - 1),
            )
        nc.scalar.activation(
            m1T[:, j, :], ps[:], AF.Gelu, bias=c1_sb[:, j : j + 1]
        )
    w1s.close()

    # ------- phase 8: FC2 (4 hid blocks) + residual -> out -------
    acc_pool = _Pool(tc, name="acc", bufs=1)
    acc = acc_pool.tile([P, NT, C], F32)
    w2s = _Pool(tc, name="w2s", bufs=2)
    ow = _Pool(tc, name="ow", bufs=2)
    c2_rep = rep_tile(ow, "c2")
    w2_r = d["w2"].rearrange("(j p) c -> p j c", p=P)
    NBLK = 4
    JB = JH // NBLK  # 8
    for blk in range(NBLK):
        w2b = w2s.tile([P, JB, C], BF16, tag="w2b", name="w2b")
        nc.scalar.dma_start(w2b[:], w2_r[:, _ts(blk, JB), :])
        for t in range(NT):
            ps = ps_tile()
            for jj in range(JB):
                j = blk * JB + jj
                lhsT = m1T[:, j, _ts(t, P)]
                nc.tensor.matmul(
                    ps[:, 0:512], lhsT, w2b[:, jj, 0:512],
                    start=(jj == 0), stop=(jj == JB - 1),
                )
                nc.tensor.matmul(
                    ps[:, 512:1024], lhsT, w2b[:, jj, 512:1024],
                    start=(jj == 0), stop=(jj == JB - 1),
                )
            if blk == 0:
                nc.vector.tensor_tensor(acc[:, t, :], ps[:], c2_rep[:], op=ALU.add)
            elif blk < NBLK - 1:
                nc.vector.tensor_tensor(
                    acc[:, t, :], acc[:, t, :], ps[:], op=ALU.add
                )
            else:
                ot = ow.tile([P, C], F32, tag="ot", name="ot")
                nc.vector.tensor_tensor(ot[:], acc[:, t, :], ps[:], op=ALU.add)
                nc.vector.tensor_tensor(ot[:], ot[:], x1[:, t, :], op=ALU.add)
                nc.sync.dma_start(d["out"][_ts(t, P), :], ot[:])
    ow.close()
    w2s.close()
    acc_pool.close()
    m1_pool.close()
    h2T_pool.close()
    x1_pool.close()
    psum.close()
    consts.close()


_CACHE = {}


def get_nc():
    key = (
        os.environ.get("KERNEL_NREP", "1"),
        bool(os.environ.get("KERNEL_DEBUG_TAPS")),
    )
    if key not in _CACHE:
        nc = bacc.Bacc(
            "TRN2", target_bir_lowering=False, debug=False, num_devices=NCORES
        )
        build_program(nc)
        nc.compile()
        _CACHE[key] = nc
    return _CACHE[key]


def make_in_maps(inputs):
    bf = lambda a: np.ascontiguousarray(np.asarray(a, np.float32)).astype(
        ml_dtypes.bfloat16
    )
    f32 = lambda a: np.ascontiguousarray(np.asarray(a, np.float32))
    shared = {
        "wq": bf(inputs["Wq"]),
        "wk": bf(inputs["Wk"]),
        "wv": bf(inputs["Wv"]),
        "wp": bf(inputs["Wp"]),
        "w1": bf(inputs["W1"]),
        "w2": bf(inputs["W2"]),
        "g1": f32(inputs["g1"]),
        "b1": f32(inputs["b1"]),
        "bq": f32(inputs["bq"]),
        "bk": f32(inputs["bk"]),
        "bv": f32(inputs["bv"]),
        "bp": f32(inputs["bp"]),
        "g2": f32(inputs["g2"]),
        "b2": f32(inputs["b2"]),
        "c1": f32(inputs["c1"]),
        "c2": f32(inputs["c2"]),
    }
    x = np.asarray(inputs["x"], np.float32)
    return [{**shared, "x": np.ascontiguousarray(x[c])} for c in range(NCORES)]


def kernel(**inputs):
    from concourse.bass_utils import run_bass_kernel_spmd

    nc = get_nc()
    in_maps = make_in_maps(inputs)
    res = run_bass_kernel_spmd(nc, in_maps, core_ids=list(range(NCORES)))
    out = np.stack(
        [np.asarray(res.results[c]["out"], np.float32) for c in range(NCORES)], axis=0
    )
    return out


# revision 44
# speedup vs baseline: 1.0073x; 1.0073x over previous
"""Trainium2 Bass kernel for an 8-batch transformer encoder block.

Strategy: pure data parallelism -- batch B=8 across 8 NeuronCores, one
batch element (1024 tokens x 1024 dim) per core, full weights on every
core, no collectives.  All matmuls run in bf16 on the TensorEngine with
f32 PSUM accumulation; LayerNorm / softmax statistics stay f32 (weights
are pre-cast to bf16 on the host).

Layout notes (per core):
  - LayerNorm runs token-major; its bf16 output is flipped to the
    feature-major [C, tokens] layout the linears need via PE transposes
    (128x128 identity matmuls, free while PE is otherwise idle).
  - q/k projections emit feature-major qT/kT (per-partition bias fused
    into the PSUM->SBUF ACT copy); v emits token-major into SBUF.
  - The reference reshapes (N, C) -> (H, N', hd) directly, so head h of
    q/k/v is the contiguous row-block [64h, 64h+64) of the projection
    reinterpreted as (1024, 64).  Per-head Q^T/K^T tiles are gathered
    with two contiguous partition-shifted SBUF->SBUF DMAs each (one per
    column parity); V chunks are strided SBUF->SBUF reads.
  - Softmax is computed on S^T (keys on partitions) with no max
    subtraction (logits are ~N(0, 0.3), |logit| < ~7, exp safe in f32).
    The denominators come free from a ones-column appended to V in the
    P@V matmul; normalization is a per-column scale of the 64-row O^T
    head output (reciprocal on DVE, partition-broadcast on GpSimd).
  - MLP: FC1 emits hidden-major m1T with exact-erf GELU + bias fused in
    the ACT PSUM->SBUF copy; FC2 accumulates over 4 hidden blocks into
    an f32 SBUF accumulator, then adds bias + residual and stores.
  - PSUM: three 2-bank [128, 1024] matmul slots (two 512-wide matmuls
    per slot, one wide ACT/DVE drain) + two transpose banks.

Measured (8 cores, NeuronCore per batch element): ~0.64 ms/block,
TimelineSim model 0.60 ms; rel_l2 vs f32 reference = 1.5e-3 (bf16
floor).  KERNEL_NREP / KERNEL_DEBUG_TAPS env vars exist for the test
harness only (timing slope / intermediate taps).
"""

import os
import sys

sys.path.insert(0, "/opt/trn_rl_repo")

import numpy as np
import ml_dtypes

import concourse.bass as bass
import concourse.tile as tile
from concourse import bacc, mybir
from concourse.masks import make_identity

B, N, C, H = 8, 1024, 1024, 16
HD = C // H  # 64
HID = 4 * C  # 4096
P = 128
NT = N // P  # token chunks
CO = C // P  # feature chunks
JH = HID // P  # hidden chunks
EPS = 1e-5

F32 = mybir.dt.float32
BF16 = mybir.dt.bfloat16
AF = mybir.ActivationFunctionType
ALU = mybir.AluOpType

NCORES = 8

WEIGHT_NAMES = ["wq", "wk", "wv", "wp", "w1", "w2"]
VEC_NAMES = ["g1", "b1", "bq", "bk", "bv", "bp", "g2", "b2", "c1", "c2"]


def _ts(i, size):
    return slice(i * size, (i + 1) * size)


class _Pool:
    """Tile pool with manually controlled (non-LIFO) lifetime."""

    def __init__(self, tc, **kw):
        self._cm = tc.tile_pool(**kw)
        self.pool = self._cm.__enter__()

    _n = 0

    def tile(self, *a, **kw):
        if "name" not in kw:
            _Pool._n += 1
            kw["name"] = f"t{_Pool._n}"
        return self.pool.tile(*a, **kw)

    def close(self):
        self._cm.__exit__(None, None, None)


def build_program(nc):
    d = {}
    d["x"] = nc.dram_tensor("x", [N, C], F32, kind="ExternalInput").ap()
    for w, shape in [
        ("wq", [C, C]),
        ("wk", [C, C]),
        ("wv", [C, C]),
        ("wp", [C, C]),
        ("w1", [C, HID]),
        ("w2", [HID, C]),
    ]:
        d[w] = nc.dram_tensor(w, shape, BF16, kind="ExternalInput").ap()
    for v in VEC_NAMES:
        size = HID if v == "c1" else C
        d[v] = nc.dram_tensor(v, [size], F32, kind="ExternalInput").ap()
    d["out"] = nc.dram_tensor("out", [N, C], F32, kind="ExternalOutput").ap()

    debug = bool(os.environ.get("KERNEL_DEBUG_TAPS"))
    dbg = {}
    if debug:
        for nm, shape, dt in [
            ("dbg_h", [N, C], BF16),
            ("dbg_v", [N, C], BF16),
            ("dbg_h2", [N, C], BF16),
            ("dbg_qT", [P, CO, N], BF16),
            ("dbg_kT", [P, CO, N], BF16),
            ("dbg_oT", [P, CO, N], BF16),
            ("dbg_x1", [P, NT, C], F32),
        ]:
            dbg[nm] = nc.dram_tensor(nm, shape, dt, kind="ExternalOutput").ap()

    nrep = int(os.environ.get("KERNEL_NREP", "1"))
    with tile.TileContext(nc) as tc:
        for rep in range(nrep):
            _emit(tc, nc, d, dbg if rep == 0 else {})
    return nc


def _emit(tc, nc, d, dbg=None):
    dbg = dbg or {}
    # ------- resident pools (level 0): tiny consts, psum, x1 -------
    consts = _Pool(tc, name="consts", bufs=1)
    bq_sb = consts.tile([P, CO], F32)
    nc.sync.dma_start(bq_sb[:], d["bq"].rearrange("(o p) -> p o", p=P))
    bk_sb = consts.tile([P, CO], F32)
    nc.sync.dma_start(bk_sb[:], d["bk"].rearrange("(o p) -> p o", p=P))
    c1_sb = consts.tile([P, JH], F32)
    nc.sync.dma_start(c1_sb[:], d["c1"].rearrange("(j p) -> p j", p=P))
    eps_sb = consts.tile([P, 1], F32)
    nc.vector.memset(eps_sb[:], EPS)
    ident = consts.tile([P, P], BF16, name="ident")
    make_identity(nc, ident[:])

    # shared PSUM pool: all psum tiles are [128, 512] f32 (one bank each)
    psum = _Pool(tc, name="psum", bufs=6, space="PSUM")

    def ps_tile():
        return psum.tile([P, N], F32, tag="mm", name="ps", bufs=3)

    def ps_tr():
        # one PSUM bank holds eight 128x128 bf16 transposes -> one wide drain
        return psum.tile([P, CO, P], BF16, tag="tr", name="pstr", bufs=2)

    x1_pool = _Pool(tc, name="x1", bufs=1)
    x1 = x1_pool.tile([P, NT, C], F32)

    def rep_tile(pool, vname):
        t = pool.tile([P, C], F32, tag=f"{vname}_rep", name=f"{vname}_rep", bufs=1)
        nc.scalar.dma_start(t[:], d[vname].partition_broadcast(P))
        return t

    # ------- LayerNorm helper -------
    def layer_norm(work, src_ap, g_rep, b_rep, t):
        st = work.tile([P, 2, 6], F32, tag="ln_st", name="st")
        nc.vector.bn_stats(st[:, 0, :], src_ap[:, 0:512])
        nc.vector.bn_stats(st[:, 1, :], src_ap[:, 512:1024])
        mv = work.tile([P, 2], F32, tag="ln_mv", name="mv")
        nc.vector.bn_aggr(mv[:], st[:])
        rstd = work.tile([P, 1], F32, tag="ln_rstd", name="rstd")
        nc.scalar.activation(rstd[:], mv[:, 1:2], AF.Sqrt, bias=eps_sb[:, :])
        nc.vector.reciprocal(rstd[:], rstd[:])
        # (x - m)*g*rstd + b fused as two scalar_tensor_tensor passes
        tmp = work.tile([P, C], F32, tag="ln_tmp", name="tmp")
        nc.vector.scalar_tensor_tensor(
            tmp[:], src_ap, mv[:, 0:1], g_rep[:], op0=ALU.subtract, op1=ALU.mult
        )
        hb = work.tile([P, C], BF16, tag="ln_out", name="hb")
        nc.vector.scalar_tensor_tensor(
            hb[:], tmp[:], rstd[:], b_rep[:], op0=ALU.mult, op1=ALU.add
        )
        return hb

    # ------- level 1 (pool stack, outermost first): h2T (lives to FC1),
    # wp/oT (live through proj), qkT (through attention), vwork (v_sb,
    # through attention), hT (through QKV) -------
    h2T_pool = _Pool(tc, name="h2T", bufs=1)
    h2T = h2T_pool.tile([P, CO, N], BF16)
    wp_pool = _Pool(tc, name="wp", bufs=1)
    wp_sb = wp_pool.tile([P, CO, C], BF16)
    oT_pool = _Pool(tc, name="oT", bufs=1)
    oT = oT_pool.tile([P, CO, N], BF16)
    qkT_pool = _Pool(tc, name="qkT", bufs=1)
    qTh = [qkT_pool.tile([P, CO, 512], BF16, name=f"qT{i}") for i in range(2)]
    kTh = [qkT_pool.tile([P, CO, 512], BF16, name=f"kT{i}") for i in range(2)]

    # v buffer lives through attention; opened before hT for LIFO order
    vwork = _Pool(tc, name="vwork", bufs=1)
    v_sb = vwork.tile([P, NT, C], BF16, name="v_sb")
    bv_rep = rep_tile(vwork, "bv")

    # ------- phase 1+2: LN1 -> PE-transpose -> hT (bf16, feature-major) ---
    hT_pool = _Pool(tc, name="hT", bufs=1)
    hT = hT_pool.tile([P, CO, N], BF16)
    ln1 = _Pool(tc, name="ln1", bufs=3)
    g1_rep = rep_tile(ln1, "g1")
    b1_rep = rep_tile(ln1, "b1")
    for t in range(NT):
        xt = ln1.tile([P, C], F32, tag="ln_x", name="xt")
        nc.sync.dma_start(xt[:], d["x"][_ts(t, P), :])
        hb = layer_norm(ln1, xt[:], g1_rep, b1_rep, t)
        ptr = ps_tr()
        for o in range(CO):
            nc.tensor.transpose(ptr[:, o, :], hb[:, _ts(o, P)], ident[:])
        nc.scalar.copy(hT[:, :, _ts(t, P)], ptr[:])

    ln1.close()

    # ------- phase 3: QKV projections -------
    wqkv = _Pool(tc, name="wqkv", bufs=1)
    w_sb = {}
    for w in ["wq", "wk", "wv"]:
        w_sb[w] = wqkv.tile([P, CO, C], BF16, name=f"{w}_sb")
        nc.sync.dma_start(w_sb[w][:], d[w].rearrange("(o p) c -> p o c", p=P))
    nc.sync.dma_start(wp_sb[:], d["wp"].rearrange("(o p) c -> p o c", p=P))

    # q/k in two token-half passes; two feature-chunks m share one wide
    # psum slot.  All q/k before v so the PE stream never stalls on the
    # later-arriving wv DMA (wq/wk land first on the weight queue).
    for half in range(2):
        hslice = slice(512 * half, 512 * (half + 1))
        for w, b_sb, dstT in (("wq", bq_sb, qTh), ("wk", bk_sb, kTh)):
            for mp in range(CO // 2):
                ps = ps_tile()
                for o in range(CO):
                    for mm in range(2):
                        m = 2 * mp + mm
                        nc.tensor.matmul(
                            ps[:, _ts(mm, 512)],
                            w_sb[w][:, o, _ts(m, P)],
                            hT[:, o, hslice],
                            start=(o == 0),
                            stop=(o == CO - 1),
                        )
                for mm in range(2):
                    m = 2 * mp + mm
                    nc.scalar.activation(
                        dstT[half][:, m, :],
                        ps[:, _ts(mm, 512)],
                        AF.Identity,
                        bias=b_sb[:, m : m + 1],
                    )
    # v token chunks after all q/k
    for t in range(NT):
        ps = ps_tile()
        for o in range(CO):
            lhsT = hT[:, o, _ts(t, P)]
            nc.tensor.matmul(
                ps[:, 0:512], lhsT, w_sb["wv"][:, o, 0:512],
                start=(o == 0), stop=(o == CO - 1),
            )
            nc.tensor.matmul(
                ps[:, 512:1024], lhsT, w_sb["wv"][:, o, 512:1024],
                start=(o == 0), stop=(o == CO - 1),
            )
        nc.vector.tensor_tensor(v_sb[:, t, :], ps[:], bv_rep[:], op=ALU.add)
    if dbg:
        for o in range(CO):
            nc.sync.dma_start(
                dbg["dbg_h"][:, _ts(o, P)].rearrange("n c -> c n"), hT[:, o, :]
            )
        nc.sync.dma_start(
            dbg["dbg_v"].rearrange("(t p) c -> p t c", p=P), v_sb[:]
        )
        for i in range(2):
            nc.sync.dma_start(dbg["dbg_qT"][:, :, _ts(i, 512)], qTh[i][:])
            nc.sync.dma_start(dbg["dbg_kT"][:, :, _ts(i, 512)], kTh[i][:])
    wqkv.close()
    hT_pool.close()

    # ------- phase 4: attention, head by head -------
    heads = _Pool(tc, name="heads", bufs=2)
    for h in range(H):
        # Q_h^T / K_h^T as [64 d, (16 beta, 64 alpha)]; attention position
        # n = 16*alpha + beta.  Source: qT[64*beta + dd, 64h + alpha].
        qh = heads.tile([HD, 16, HD], BF16, tag="qh", name="qh")
        kh = heads.tile([HD, 16, HD], BF16, tag="kh", name="kh")
        hv = h % 8  # token offset within the half tile
        for srcT, dstT in ((qTh[h // 8], qh), (kTh[h // 8], kh)):
            # all b of one parity in a single DMA: b = 2o + bb
            for bb in range(2):
                nc.sync.dma_start(
                    dstT[:, bb::2, :],
                    srcT[64 * bb : 64 * bb + HD, :, _ts(hv, HD)],
                )
        # V_h chunks + ones column for softmax denominators.  Chunk i holds
        # m-values with m%16 in {2i, 2i+1} at partition p = 64*bb + a'
        # (m = 16a' + 2i + bb), matching the S^T psum partition order below.
        vh = heads.tile([P, 8, HD + 1], BF16, tag="vh", name="vh")
        nc.vector.memset(vh[:, :, HD : HD + 1], 1.0)
        # v rows 64h..64h+64 live at partitions 64*(h%2).. of chunk h//2
        vrow = v_sb[64 * (h % 2) : 64 * (h % 2) + 64, h // 2, :].rearrange(
            "t (g dd) -> t g dd", dd=HD
        )
        for bb in range(2):
            nc.sync.dma_start(
                vh[64 * bb : 64 * bb + 64, :, 0:HD], vrow[:, bb::2, :]
            )

        # S^T = K_h Q_h^T (keys on partitions), exp via ACT (scale=1/8).
        # psum partition p = 64*(b'%2) + a' <-> m = 16a' + 2i + b'%2.
        est = heads.tile([P, 8, N], BF16, tag="est", name="est")
        for i in range(8):
            ps = ps_tile()
            lhsT = kh[:, 2 * i : 2 * i + 2, :]  # [64, 128]
            nc.tensor.matmul(
                ps[:, 0:512], lhsT, qh[:, 0:8, :], start=True, stop=True
            )
            nc.tensor.matmul(
                ps[:, 512:1024], lhsT, qh[:, 8:16, :], start=True, stop=True
            )
            nc.scalar.activation(est[:, i, :], ps[:], AF.Exp, scale=0.125)

        # O^T = [V|1]^T expS^T : rows 0..63 head output, row 64 denominators
        po = ps_tile()
        for i in range(8):
            nc.tensor.matmul(
                po[0 : HD + 1, 0:512],
                vh[:, i, :],
                est[:, i, 0:512],
                start=(i == 0),
                stop=(i == 7),
            )
            nc.tensor.matmul(
                po[0 : HD + 1, 512:1024],
                vh[:, i, :],
                est[:, i, 512:1024],
                start=(i == 0),
                stop=(i == 7),
            )
        r = heads.tile([1, N], F32, tag="r", name="r")
        nc.vector.reciprocal(r[:], po[HD : HD + 1, :])
        rr = heads.tile([HD, N], F32, tag="rr", name="rr")
        nc.gpsimd.partition_broadcast(rr[:], r[:], channels=HD)

        # normalize + un-permute (beta, alpha) -> n = 16*alpha + beta
        p0 = HD * (h % 2)
        oc = h // 2
        for half in range(2):
            dst = oT[p0 : p0 + HD, oc, :].rearrange("p (a b2) -> p b2 a", b2=16)[
                :, 8 * half : 8 * half + 8, :
            ]
            src_ps = po[0:HD, _ts(half, 512)].rearrange("p (b2 a) -> p b2 a", b2=8)
            src_rr = rr[:, _ts(half, 512)].rearrange("p (b2 a) -> p b2 a", b2=8)
            nc.vector.tensor_tensor(dst, src_ps, src_rr, op=ALU.mult)
    heads.close()
    vwork.close()
    qkT_pool.close()
    if dbg:
        nc.sync.dma_start(dbg["dbg_oT"], oT[:])

    # ------- phase 5: proj + residual -> x1 ; LN2 -> h2T (PE transpose) ---
    ln2 = _Pool(tc, name="ln2", bufs=3)
    g2_rep = rep_tile(ln2, "g2")
    b2_rep = rep_tile(ln2, "b2")
    bp_rep = rep_tile(ln2, "bp")
    for t in range(NT):
        ps = ps_tile()
        for o in range(CO):
            lhsT = oT[:, o, _ts(t, P)]
            nc.tensor.matmul(
                ps[:, 0:512], lhsT, wp_sb[:, o, 0:512],
                start=(o == 0), stop=(o == CO - 1),
            )
            nc.tensor.matmul(
                ps[:, 512:1024], lhsT, wp_sb[:, o, 512:1024],
                start=(o == 0), stop=(o == CO - 1),
            )
        xt = ln2.tile([P, C], F32, tag="ln_x", name="xt")
        nc.sync.dma_start(xt[:], d["x"][_ts(t, P), :])
        nc.vector.tensor_tensor(x1[:, t, :], ps[:], bp_rep[:], op=ALU.add)
        nc.vector.tensor_tensor(x1[:, t, :], x1[:, t, :], xt[:], op=ALU.add)
        hb2 = layer_norm(ln2, x1[:, t, :], g2_rep, b2_rep, t)
        ptr = ps_tr()
        for o in range(CO):
            nc.tensor.transpose(ptr[:, o, :], hb2[:, _ts(o, P)], ident[:])
        nc.scalar.copy(h2T[:, :, _ts(t, P)], ptr[:])
    ln2.close()
    oT_pool.close()
    wp_pool.close()
    if dbg:
        nc.sync.dma_start(dbg["dbg_x1"], x1[:])
        for o in range(CO):
            nc.sync.dma_start(
                dbg["dbg_h2"][:, _ts(o, P)].rearrange("n c -> c n"), h2T[:, o, :]
            )

    # ------- phase 7: FC1 + exact GELU -> m1T -------
    m1_pool = _Pool(tc, name="m1T", bufs=1)
    m1T = m1_pool.tile([P, JH, N], BF16)
    w1s = _Pool(tc, name="w1s", bufs=3)
    w1_r = d["w1"].rearrange("(o p) c -> p o c", p=P)
    for j in range(JH):
        w1t = w1s.tile([P, CO, P], BF16, tag="w1t", name="w1t")
        nc.scalar.dma_start(w1t[:], w1_r[:, :, _ts(j, P)])
        ps = ps_tile()
        for o in range(CO):
            nc.tensor.matmul(
                ps[:, 0:512], w1t[:, o, :], h2T[:, o, 0:512],
                start=(o == 0), stop=(o == CO - 1),
            )
            nc.tensor.matmul(
                ps[:, 512:1024], w1t[:, o, :], h2T[:, o, 512:1024],
                start=(o == 0), stop=(o == CO - 1),
            )
        nc.scalar.activation(
            m1T[:, j, :], ps[:], AF.Gelu, bias=c1_sb[:, j : j + 1]
        )
    w1s.close()

    # ------- phase 8: FC2 (4 hid blocks) + residual -> out -------
    acc_pool = _Pool(tc, name="acc", bufs=1)
    acc = acc_pool.tile([P, NT, C], F32)
    w2s = _Pool(tc, name="w2s", bufs=2)
    ow = _Pool(tc, name="ow", bufs=2)
    c2_rep = rep_tile(ow, "c2")
    w2_r = d["w2"].rearrange("(j p) c -> p j c", p=P)
    NBLK = 4
    JB = JH // NBLK  # 8
    for blk in range(NBLK):
        w2b = w2s.tile([P, JB, C], BF16, tag="w2b", name="w2b")
        nc.scalar.dma_start(w2b[:], w2_r[:, _ts(blk, JB), :])
        for t in range(NT):
            ps = ps_tile()
            for jj in range(JB):
                j = blk * JB + jj
                lhsT = m1T[:, j, _ts(t, P)]
                nc.tensor.matmul(
                    ps[:, 0:512], lhsT, w2b[:, jj, 0:512],
                    start=(jj == 0), stop=(jj == JB - 1),
                )
                nc.tensor.matmul(
                    ps[:, 512:1024], lhsT, w2b[:, jj, 512:1024],
                    start=(jj == 0), stop=(jj == JB - 1),
                )
            if blk == 0:
                nc.vector.tensor_tensor(acc[:, t, :], ps[:], c2_rep[:], op=ALU.add)
            elif blk < NBLK - 1:
                nc.vector.tensor_tensor(
                    acc[:, t, :], acc[:, t, :], ps[:], op=ALU.add
                )
            else:
                ot = ow.tile([P, C], F32, tag="ot", name="ot")
                nc.vector.tensor_tensor(ot[:], acc[:, t, :], ps[:], op=ALU.add)
                nc.vector.tensor_tensor(ot[:], ot[:], x1[:, t, :], op=ALU.add)
                nc.sync.dma_start(d["out"][_ts(t, P), :], ot[:])
    ow.close()
    w2s.close()
    acc_pool.close()
    m1_pool.close()
    h2T_pool.close()
    x1_pool.close()
    psum.close()
    consts.close()


_CACHE = {}


def get_nc():
    key = (
        os.environ.get("KERNEL_NREP", "1"),
        bool(os.environ.get("KERNEL_DEBUG_TAPS")),
    )
    if key not in _CACHE:
        nc = bacc.Bacc(
            "TRN2", target_bir_lowering=False, debug=False, num_devices=NCORES
        )
        build_program(nc)
        nc.compile()
        _CACHE[key] = nc
    return _CACHE[key]


def make_in_maps(inputs):
    bf = lambda a: np.ascontiguousarray(np.asarray(a, np.float32)).astype(
        ml_dtypes.bfloat16
    )
    f32 = lambda a: np.ascontiguousarray(np.asarray(a, np.float32))
    shared = {
        "wq": bf(inputs["Wq"]),
        "wk": bf(inputs["Wk"]),
        "wv": bf(inputs["Wv"]),
        "wp": bf(inputs["Wp"]),
        "w1": bf(inputs["W1"]),
        "w2": bf(inputs["W2"]),
        "g1": f32(inputs["g1"]),
        "b1": f32(inputs["b1"]),
        "bq": f32(inputs["bq"]),
        "bk": f32(inputs["bk"]),
        "bv": f32(inputs["bv"]),
        "bp": f32(inputs["bp"]),
        "g2": f32(inputs["g2"]),
        "b2": f32(inputs["b2"]),
        "c1": f32(inputs["c1"]),
        "c2": f32(inputs["c2"]),
    }
    x = np.asarray(inputs["x"], np.float32)
    return [{**shared, "x": np.ascontiguousarray(x[c])} for c in range(NCORES)]


def kernel(**inputs):
    from concourse.bass_utils import run_bass_kernel_spmd

    nc = get_nc()
    in_maps = make_in_maps(inputs)
    res = run_bass_kernel_spmd(nc, in_maps, core_ids=list(range(NCORES)))
    out = np.stack(
        [np.asarray(res.results[c]["out"], np.float32) for c in range(NCORES)], axis=0
    )
    return out


# revision 45
# speedup vs baseline: 1.0832x; 1.0754x over previous
"""Trainium2 Bass kernel for an 8-batch transformer encoder block.

Strategy: pure data parallelism -- batch B=8 across 8 NeuronCores, one
batch element (1024 tokens x 1024 dim) per core, full weights on every
core, no collectives.  All matmuls run in bf16 on the TensorEngine with
f32 PSUM accumulation; LayerNorm / softmax statistics stay f32 (weights
are pre-cast to bf16 on the host).

Layout notes (per core):
  - LayerNorm runs token-major; its bf16 output is flipped to the
    feature-major [C, tokens] layout the linears need via PE transposes
    (128x128 identity matmuls, free while PE is otherwise idle).
  - q/k projections emit feature-major qT/kT (per-partition bias fused
    into the PSUM->SBUF ACT copy); v emits token-major into SBUF.
  - The reference reshapes (N, C) -> (H, N', hd) directly, so head h of
    q/k/v is the contiguous row-block [64h, 64h+64) of the projection
    reinterpreted as (1024, 64).  Per-head Q^T/K^T tiles are gathered
    with two contiguous partition-shifted SBUF->SBUF DMAs each (one per
    column parity); V chunks are strided SBUF->SBUF reads.
  - Softmax is computed on S^T (keys on partitions) with no max
    subtraction (logits are ~N(0, 0.3), |logit| < ~7, exp safe in f32).
    The denominators come free from a ones-column appended to V in the
    P@V matmul; normalization is a per-column scale of the 64-row O^T
    head output (reciprocal on DVE, partition-broadcast on GpSimd).
  - MLP: FC1 emits hidden-major m1T with exact-erf GELU + bias fused in
    the ACT PSUM->SBUF copy; FC2 accumulates over 4 hidden blocks into
    an f32 SBUF accumulator, then adds bias + residual and stores.
  - PSUM: three 2-bank [128, 1024] matmul slots (two 512-wide matmuls
    per slot, one wide ACT/DVE drain) + two transpose banks.

Measured (8 cores, NeuronCore per batch element): ~0.64 ms/block,
TimelineSim model 0.60 ms; rel_l2 vs f32 reference = 1.5e-3 (bf16
floor).  KERNEL_NREP / KERNEL_DEBUG_TAPS env vars exist for the test
harness only (timing slope / intermediate taps).
"""

import os
import sys

sys.path.insert(0, "/opt/trn_rl_repo")

import numpy as np
import ml_dtypes

import concourse.bass as bass
import concourse.tile as tile
from concourse import bacc, mybir
from concourse.masks import make_identity

B, N, C, H = 8, 1024, 1024, 16
HD = C // H  # 64
HID = 4 * C  # 4096
P = 128
NT = N // P  # token chunks
CO = C // P  # feature chunks
JH = HID // P  # hidden chunks
EPS = 1e-5

F32 = mybir.dt.float32
BF16 = mybir.dt.bfloat16
AF = mybir.ActivationFunctionType
ALU = mybir.AluOpType

NCORES = 8

WEIGHT_NAMES = ["wq", "wk", "wv", "wp", "w1", "w2"]
VEC_NAMES = ["g1", "b1", "bq", "bk", "bv", "bp", "g2", "b2", "c1", "c2"]


def _ts(i, size):
    return slice(i * size, (i + 1) * size)


class _Pool:
    """Tile pool with manually controlled (non-LIFO) lifetime."""

    def __init__(self, tc, **kw):
        self._cm = tc.tile_pool(**kw)
        self.pool = self._cm.__enter__()

    _n = 0

    def tile(self, *a, **kw):
        if "name" not in kw:
            _Pool._n += 1
            kw["name"] = f"t{_Pool._n}"
        return self.pool.tile(*a, **kw)

    def close(self):
        self._cm.__exit__(None, None, None)


def build_program(nc):
    d = {}
    d["x"] = nc.dram_tensor("x", [N, C], F32, kind="ExternalInput").ap()
    for w, shape in [
        ("wq", [C, C]),
        ("wk", [C, C]),
        ("wv", [C, C]),
        ("wp", [C, C]),
        ("w1", [C, HID]),
        ("w2", [HID, C]),
    ]:
        d[w] = nc.dram_tensor(w, shape, BF16, kind="ExternalInput").ap()
    for v in VEC_NAMES:
        size = HID if v == "c1" else C
        d[v] = nc.dram_tensor(v, [size], F32, kind="ExternalInput").ap()
    d["out"] = nc.dram_tensor("out", [N, C], F32, kind="ExternalOutput").ap()

    debug = bool(os.environ.get("KERNEL_DEBUG_TAPS"))
    dbg = {}
    if debug:
        for nm, shape, dt in [
            ("dbg_h", [N, C], BF16),
            ("dbg_v", [N, C], BF16),
            ("dbg_h2", [N, C], BF16),
            ("dbg_qT", [P, CO, N], BF16),
            ("dbg_kT", [P, CO, N], BF16),
            ("dbg_oT", [P, CO, N], BF16),
            ("dbg_x1", [P, NT, C], F32),
        ]:
            dbg[nm] = nc.dram_tensor(nm, shape, dt, kind="ExternalOutput").ap()

    nrep = int(os.environ.get("KERNEL_NREP", "1"))
    with tile.TileContext(nc) as tc:
        for rep in range(nrep):
            _emit(tc, nc, d, dbg if rep == 0 else {})
    return nc


def _emit(tc, nc, d, dbg=None):
    dbg = dbg or {}
    # ------- resident pools (level 0): tiny consts, psum, x1 -------
    consts = _Pool(tc, name="consts", bufs=1)
    bq_sb = consts.tile([P, CO], F32)
    nc.sync.dma_start(bq_sb[:], d["bq"].rearrange("(o p) -> p o", p=P))
    bk_sb = consts.tile([P, CO], F32)
    nc.sync.dma_start(bk_sb[:], d["bk"].rearrange("(o p) -> p o", p=P))
    c1_sb = consts.tile([P, JH], F32)
    nc.sync.dma_start(c1_sb[:], d["c1"].rearrange("(j p) -> p j", p=P))
    eps_sb = consts.tile([P, 1], F32)
    nc.vector.memset(eps_sb[:], EPS)
    ident = consts.tile([P, P], BF16, name="ident")
    make_identity(nc, ident[:])

    # shared PSUM pool: all psum tiles are [128, 512] f32 (one bank each)
    psum = _Pool(tc, name="psum", bufs=6, space="PSUM")

    def ps_tile():
        return psum.tile([P, N], F32, tag="mm", name="ps", bufs=3)

    def ps_tr():
        # one PSUM bank holds eight 128x128 bf16 transposes -> one wide drain
        return psum.tile([P, CO, P], BF16, tag="tr", name="pstr", bufs=2)

    x1_pool = _Pool(tc, name="x1", bufs=1)
    x1 = x1_pool.tile([P, NT, C], F32)

    def rep_tile(pool, vname):
        t = pool.tile([P, C], F32, tag=f"{vname}_rep", name=f"{vname}_rep", bufs=1)
        nc.scalar.dma_start(t[:], d[vname].partition_broadcast(P))
        return t

    # ------- LayerNorm helper -------
    def layer_norm(work, src_ap, g_rep, b_rep, t):
        st = work.tile([P, 2, 6], F32, tag="ln_st", name="st")
        nc.vector.bn_stats(st[:, 0, :], src_ap[:, 0:512])
        nc.vector.bn_stats(st[:, 1, :], src_ap[:, 512:1024])
        mv = work.tile([P, 2], F32, tag="ln_mv", name="mv")
        nc.vector.bn_aggr(mv[:], st[:])
        rstd = work.tile([P, 1], F32, tag="ln_rstd", name="rstd")
        nc.scalar.activation(rstd[:], mv[:, 1:2], AF.Sqrt, bias=eps_sb[:, :])
        nc.vector.reciprocal(rstd[:], rstd[:])
        # (x - m)*g*rstd + b fused as two scalar_tensor_tensor passes
        tmp = work.tile([P, C], F32, tag="ln_tmp", name="tmp")
        nc.vector.scalar_tensor_tensor(
            tmp[:], src_ap, mv[:, 0:1], g_rep[:], op0=ALU.subtract, op1=ALU.mult
        )
        hb = work.tile([P, C], BF16, tag="ln_out", name="hb")
        nc.vector.scalar_tensor_tensor(
            hb[:], tmp[:], rstd[:], b_rep[:], op0=ALU.mult, op1=ALU.add
        )
        return hb

    # ------- level 1 (pool stack, outermost first): h2T (lives to FC1),
    # wp/oT (live through proj), qkT (through attention), vwork (v_sb,
    # through attention), hT (through QKV) -------
    h2T_pool = _Pool(tc, name="h2T", bufs=1)
    h2T = h2T_pool.tile([P, CO, N], BF16)
    wp_pool = _Pool(tc, name="wp", bufs=1)
    wp_sb = wp_pool.tile([P, CO, C], BF16)
    oT_pool = _Pool(tc, name="oT", bufs=1)
    oT = oT_pool.tile([P, CO, N], BF16)
    qkT_pool = _Pool(tc, name="qkT", bufs=1)
    qTh = [qkT_pool.tile([P, CO, 512], BF16, name=f"qT{i}") for i in range(2)]
    kTh = [qkT_pool.tile([P, CO, 512], BF16, name=f"kT{i}") for i in range(2)]

    # v buffer lives through attention; opened before hT for LIFO order
    vwork = _Pool(tc, name="vwork", bufs=1)
    v_sb = vwork.tile([P, NT, C], BF16, name="v_sb")
    bv_rep = rep_tile(vwork, "bv")

    # ------- phase 1+2: LN1 -> PE-transpose -> hT (bf16, feature-major) ---
    hT_pool = _Pool(tc, name="hT", bufs=1)
    hT = hT_pool.tile([P, CO, N], BF16)
    ln1 = _Pool(tc, name="ln1", bufs=3)
    g1_rep = rep_tile(ln1, "g1")
    b1_rep = rep_tile(ln1, "b1")
    for t in range(NT):
        xt = ln1.tile([P, C], F32, tag="ln_x", name="xt")
        with tc.high_priority():
            nc.sync.dma_start(xt[:], d["x"][_ts(t, P), :])
        hb = layer_norm(ln1, xt[:], g1_rep, b1_rep, t)
        ptr = ps_tr()
        for o in range(CO):
            nc.tensor.transpose(ptr[:, o, :], hb[:, _ts(o, P)], ident[:])
        nc.scalar.copy(hT[:, :, _ts(t, P)], ptr[:])

    ln1.close()

    # ------- phase 3: QKV projections -------
    wqkv = _Pool(tc, name="wqkv", bufs=1)
    w_sb = {}
    for w in ["wq", "wk", "wv"]:
        w_sb[w] = wqkv.tile([P, CO, C], BF16, name=f"{w}_sb")
        nc.sync.dma_start(w_sb[w][:], d[w].rearrange("(o p) c -> p o c", p=P))
    nc.sync.dma_start(wp_sb[:], d["wp"].rearrange("(o p) c -> p o c", p=P))

    # q/k in two token-half passes; two feature-chunks m share one wide
    # psum slot.  All q/k before v so the PE stream never stalls on the
    # later-arriving wv DMA (wq/wk land first on the weight queue).
    for half in range(2):
        hslice = slice(512 * half, 512 * (half + 1))
        for w, b_sb, dstT in (("wq", bq_sb, qTh), ("wk", bk_sb, kTh)):
            for mp in range(CO // 2):
                ps = ps_tile()
                for o in range(CO):
                    for mm in range(2):
                        m = 2 * mp + mm
                        nc.tensor.matmul(
                            ps[:, _ts(mm, 512)],
                            w_sb[w][:, o, _ts(m, P)],
                            hT[:, o, hslice],
                            start=(o == 0),
                            stop=(o == CO - 1),
                        )
                for mm in range(2):
                    m = 2 * mp + mm
                    nc.scalar.activation(
                        dstT[half][:, m, :],
                        ps[:, _ts(mm, 512)],
                        AF.Identity,
                        bias=b_sb[:, m : m + 1],
                    )
    # v token chunks after all q/k
    for t in range(NT):
        ps = ps_tile()
        for o in range(CO):
            lhsT = hT[:, o, _ts(t, P)]
            nc.tensor.matmul(
                ps[:, 0:512], lhsT, w_sb["wv"][:, o, 0:512],
                start=(o == 0), stop=(o == CO - 1),
            )
            nc.tensor.matmul(
                ps[:, 512:1024], lhsT, w_sb["wv"][:, o, 512:1024],
                start=(o == 0), stop=(o == CO - 1),
            )
        nc.vector.tensor_tensor(v_sb[:, t, :], ps[:], bv_rep[:], op=ALU.add)
    if dbg:
        for o in range(CO):
            nc.sync.dma_start(
                dbg["dbg_h"][:, _ts(o, P)].rearrange("n c -> c n"), hT[:, o, :]
            )
        nc.sync.dma_start(
            dbg["dbg_v"].rearrange("(t p) c -> p t c", p=P), v_sb[:]
        )
        for i in range(2):
            nc.sync.dma_start(dbg["dbg_qT"][:, :, _ts(i, 512)], qTh[i][:])
            nc.sync.dma_start(dbg["dbg_kT"][:, :, _ts(i, 512)], kTh[i][:])
    wqkv.close()
    hT_pool.close()

    # ------- phase 4: attention, head by head -------
    heads = _Pool(tc, name="heads", bufs=2)
    for h in range(H):
        # Q_h^T / K_h^T as [64 d, (16 beta, 64 alpha)]; attention position
        # n = 16*alpha + beta.  Source: qT[64*beta + dd, 64h + alpha].
        qh = heads.tile([HD, 16, HD], BF16, tag="qh", name="qh")
        kh = heads.tile([HD, 16, HD], BF16, tag="kh", name="kh")
        hv = h % 8  # token offset within the half tile
        for srcT, dstT in ((qTh[h // 8], qh), (kTh[h // 8], kh)):
            # all b of one parity in a single DMA: b = 2o + bb
            for bb in range(2):
                nc.sync.dma_start(
                    dstT[:, bb::2, :],
                    srcT[64 * bb : 64 * bb + HD, :, _ts(hv, HD)],
                )
        # V_h chunks + ones column for softmax denominators.  Chunk i holds
        # m-values with m%16 in {2i, 2i+1} at partition p = 64*bb + a'
        # (m = 16a' + 2i + bb), matching the S^T psum partition order below.
        vh = heads.tile([P, 8, HD + 1], BF16, tag="vh", name="vh")
        nc.vector.memset(vh[:, :, HD : HD + 1], 1.0)
        # v rows 64h..64h+64 live at partitions 64*(h%2).. of chunk h//2
        vrow = v_sb[64 * (h % 2) : 64 * (h % 2) + 64, h // 2, :].rearrange(
            "t (g dd) -> t g dd", dd=HD
        )
        for bb in range(2):
            nc.sync.dma_start(
                vh[64 * bb : 64 * bb + 64, :, 0:HD], vrow[:, bb::2, :]
            )

        # S^T = K_h Q_h^T (keys on partitions), exp via ACT (scale=1/8).
        # psum partition p = 64*(b'%2) + a' <-> m = 16a' + 2i + b'%2.
        est = heads.tile([P, 8, N], BF16, tag="est", name="est")
        for i in range(8):
            ps = ps_tile()
            lhsT = kh[:, 2 * i : 2 * i + 2, :]  # [64, 128]
            nc.tensor.matmul(
                ps[:, 0:512], lhsT, qh[:, 0:8, :], start=True, stop=True
            )
            nc.tensor.matmul(
                ps[:, 512:1024], lhsT, qh[:, 8:16, :], start=True, stop=True
            )
            nc.scalar.activation(est[:, i, :], ps[:], AF.Exp, scale=0.125)

        # O^T = [V|1]^T expS^T : rows 0..63 head output, row 64 denominators
        po = ps_tile()
        for i in range(8):
            nc.tensor.matmul(
                po[0 : HD + 1, 0:512],
                vh[:, i, :],
                est[:, i, 0:512],
                start=(i == 0),
                stop=(i == 7),
            )
            nc.tensor.matmul(
                po[0 : HD + 1, 512:1024],
                vh[:, i, :],
                est[:, i, 512:1024],
                start=(i == 0),
                stop=(i == 7),
            )
        r = heads.tile([1, N], F32, tag="r", name="r")
        nc.vector.reciprocal(r[:], po[HD : HD + 1, :])
        rr = heads.tile([HD, N], F32, tag="rr", name="rr")
        nc.gpsimd.partition_broadcast(rr[:], r[:], channels=HD)

        # normalize + un-permute (beta, alpha) -> n = 16*alpha + beta
        p0 = HD * (h % 2)
        oc = h // 2
        for half in range(2):
            dst = oT[p0 : p0 + HD, oc, :].rearrange("p (a b2) -> p b2 a", b2=16)[
                :, 8 * half : 8 * half + 8, :
            ]
            src_ps = po[0:HD, _ts(half, 512)].rearrange("p (b2 a) -> p b2 a", b2=8)
            src_rr = rr[:, _ts(half, 512)].rearrange("p (b2 a) -> p b2 a", b2=8)
            nc.vector.tensor_tensor(dst, src_ps, src_rr, op=ALU.mult)
    heads.close()
    vwork.close()
    qkT_pool.close()
    if dbg:
        nc.sync.dma_start(dbg["dbg_oT"], oT[:])

    # ------- phase 5: proj + residual -> x1 ; LN2 -> h2T (PE transpose) ---
    ln2 = _Pool(tc, name="ln2", bufs=3)
    g2_rep = rep_tile(ln2, "g2")
    b2_rep = rep_tile(ln2, "b2")
    bp_rep = rep_tile(ln2, "bp")
    for t in range(NT):
        ps = ps_tile()
        for o in range(CO):
            lhsT = oT[:, o, _ts(t, P)]
            nc.tensor.matmul(
                ps[:, 0:512], lhsT, wp_sb[:, o, 0:512],
                start=(o == 0), stop=(o == CO - 1),
            )
            nc.tensor.matmul(
                ps[:, 512:1024], lhsT, wp_sb[:, o, 512:1024],
                start=(o == 0), stop=(o == CO - 1),
            )
        xt = ln2.tile([P, C], F32, tag="ln_x", name="xt")
        nc.sync.dma_start(xt[:], d["x"][_ts(t, P), :])
        nc.vector.tensor_tensor(x1[:, t, :], ps[:], bp_rep[:], op=ALU.add)
        nc.vector.tensor_tensor(x1[:, t, :], x1[:, t, :], xt[:], op=ALU.add)
        hb2 = layer_norm(ln2, x1[:, t, :], g2_rep, b2_rep, t)
        ptr = ps_tr()
        for o in range(CO):
            nc.tensor.transpose(ptr[:, o, :], hb2[:, _ts(o, P)], ident[:])
        nc.scalar.copy(h2T[:, :, _ts(t, P)], ptr[:])
    ln2.close()
    oT_pool.close()
    wp_pool.close()
    if dbg:
        nc.sync.dma_start(dbg["dbg_x1"], x1[:])
        for o in range(CO):
            nc.sync.dma_start(
                dbg["dbg_h2"][:, _ts(o, P)].rearrange("n c -> c n"), h2T[:, o, :]
            )

    # ------- phase 7: FC1 + exact GELU -> m1T -------
    m1_pool = _Pool(tc, name="m1T", bufs=1)
    m1T = m1_pool.tile([P, JH, N], BF16)
    w1s = _Pool(tc, name="w1s", bufs=3)
    w1_r = d["w1"].rearrange("(o p) c -> p o c", p=P)
    for j in range(JH):
        w1t = w1s.tile([P, CO, P], BF16, tag="w1t", name="w1t")
        nc.scalar.dma_start(w1t[:], w1_r[:, :, _ts(j, P)])
        ps = ps_tile()
        for o in range(CO):
            nc.tensor.matmul(
                ps[:, 0:512], w1t[:, o, :], h2T[:, o, 0:512],
                start=(o == 0), stop=(o == CO - 1),
            )
            nc.tensor.matmul(
                ps[:, 512:1024], w1t[:, o, :], h2T[:, o, 512:1024],
                start=(o == 0), stop=(o == CO - 1),
            )
        nc.scalar.activation(
            m1T[:, j, :], ps[:], AF.Gelu, bias=c1_sb[:, j : j + 1]
        )
    w1s.close()

    # ------- phase 8: FC2 (4 hid blocks) + residual -> out -------
    acc_pool = _Pool(tc, name="acc", bufs=1)
    acc = acc_pool.tile([P, NT, C], F32)
    w2s = _Pool(tc, name="w2s", bufs=2)
    ow = _Pool(tc, name="ow", bufs=2)
    c2_rep = rep_tile(ow, "c2")
    w2_r = d["w2"].rearrange("(j p) c -> p j c", p=P)
    NBLK = 4
    JB = JH // NBLK  # 8
    for blk in range(NBLK):
        w2b = w2s.tile([P, JB, C], BF16, tag="w2b", name="w2b")
        nc.scalar.dma_start(w2b[:], w2_r[:, _ts(blk, JB), :])
        for t in range(NT):
            ps = ps_tile()
            for jj in range(JB):
                j = blk * JB + jj
                lhsT = m1T[:, j, _ts(t, P)]
                nc.tensor.matmul(
                    ps[:, 0:512], lhsT, w2b[:, jj, 0:512],
                    start=(jj == 0), stop=(jj == JB - 1),
                )
                nc.tensor.matmul(
                    ps[:, 512:1024], lhsT, w2b[:, jj, 512:1024],
                    start=(jj == 0), stop=(jj == JB - 1),
                )
            if blk == 0:
                nc.vector.tensor_tensor(acc[:, t, :], ps[:], c2_rep[:], op=ALU.add)
            elif blk < NBLK - 1:
                nc.vector.tensor_tensor(
                    acc[:, t, :], acc[:, t, :], ps[:], op=ALU.add
                )
            else:
                ot = ow.tile([P, C], F32, tag="ot", name="ot")
                nc.vector.tensor_tensor(ot[:], acc[:, t, :], ps[:], op=ALU.add)
                nc.vector.tensor_tensor(ot[:], ot[:], x1[:, t, :], op=ALU.add)
                nc.sync.dma_start(d["out"][_ts(t, P), :], ot[:])
    ow.close()
    w2s.close()
    acc_pool.close()
    m1_pool.close()
    h2T_pool.close()
    x1_pool.close()
    psum.close()
    consts.close()


_CACHE = {}


def get_nc():
    key = (
        os.environ.get("KERNEL_NREP", "1"),
        bool(os.environ.get("KERNEL_DEBUG_TAPS")),
    )
    if key not in _CACHE:
        nc = bacc.Bacc(
            "TRN2", target_bir_lowering=False, debug=False, num_devices=NCORES
        )
        build_program(nc)
        nc.compile()
        _CACHE[key] = nc
    return _CACHE[key]


def make_in_maps(inputs):
    bf = lambda a: np.ascontiguousarray(np.asarray(a, np.float32)).astype(
        ml_dtypes.bfloat16
    )
    f32 = lambda a: np.ascontiguousarray(np.asarray(a, np.float32))
    shared = {
        "wq": bf(inputs["Wq"]),
        "wk": bf(inputs["Wk"]),
        "wv": bf(inputs["Wv"]),
        "wp": bf(inputs["Wp"]),
        "w1": bf(inputs["W1"]),
        "w2": bf(inputs["W2"]),
        "g1": f32(inputs["g1"]),
        "b1": f32(inputs["b1"]),
        "bq": f32(inputs["bq"]),
        "bk": f32(inputs["bk"]),
        "bv": f32(inputs["bv"]),
        "bp": f32(inputs["bp"]),
        "g2": f32(inputs["g2"]),
        "b2": f32(inputs["b2"]),
        "c1": f32(inputs["c1"]),
        "c2": f32(inputs["c2"]),
    }
    x = np.asarray(inputs["x"], np.float32)
    return [{**shared, "x": np.ascontiguousarray(x[c])} for c in range(NCORES)]


def kernel(**inputs):
    from concourse.bass_utils import run_bass_kernel_spmd

    nc = get_nc()
    in_maps = make_in_maps(inputs)
    res = run_bass_kernel_spmd(nc, in_maps, core_ids=list(range(NCORES)))
    out = np.stack(
        [np.asarray(res.results[c]["out"], np.float32) for c in range(NCORES)], axis=0
    )
    return out
